# Initial kernel scaffold
#
"""Trainium2 Bass kernel for a BERT-style transformer encoder block.

Problem: x[2,2048,768] -> attention(12 heads) + FFN(3072) block, f32 in/out.

Sharding (8 cores): sequence-parallel. Core c handles batch b=c//4 and query
rows qi=c%4 (512 rows). Each core computes K^T/V for its WHOLE batch
(duplicated 4x within the batch group — measured cheaper than an AllGather
on this fabric), does attention for its 512 queries over all 2048 keys,
then proj+LN+FFN+LN row-parallel. No collectives.

Key layout/schedule choices (compute bf16 on TensorE, f32 accumulate):
- Q^T/K^T stored [128part=dout-chunk, 6, q/k]; per-head [64,*] slices give
  natural lhsT/rhs for S^T = K @ Q^T. Head PAIRS share a 128-partition tile,
  so the two S^T matmuls use row-groups 0/64 concurrently (tile_position).
- softmax without max-subtraction (scores are O(1)); exp on ScalarE with
  the 1/sqrt(hd) folded into the activation scale; denominators via a
  mask-broadcast lhsT matmul into a second PSUM tile (col-group packing).
- attention mask folded multiplicatively into V and the denominator lhsT.
- P@V as h^T = V^T @ P^T with natural-layout V as lhsT (no transposes).
- K^T/V production for the NEXT head-pair superstep is emitted as filler
  thunks inside the attention kc-loop: PE never idles while ScalarE exps,
  and stays HAM-warm at 2.4 GHz.
- LN via bn_stats/bn_aggr; rstd = exp(-0.5*ln(var+eps)) to stay in the
  natural_log_exp activation table set (shared with attention's exp).
"""

import numpy as np
import ml_dtypes

import concourse.bass as bass
import concourse.mybir as mybir
import concourse.tile as tile
from concourse.masks import make_identity

BF = mybir.dt.bfloat16
F32 = mybir.dt.float32
AF = mybir.ActivationFunctionType
ALU = mybir.AluOpType

B, S, D, DFF, H, HD = 2, 2048, 768, 3072, 12, 64
NCORES = 8
QW = 512            # query rows per core
DK = D // 128       # 6 chunks of the model dim
DT = DFF // 128     # 24 chunks of the ffn dim
KC = S // 128       # 16 key chunks
RT = QW // 128      # 4 row tiles per core
NP = H // 2         # 6 head pairs
EPS = 1e-12

_cached = {}


def _split_sync_waits(nc, maxw=1):
    """This walrus build supports only ONE sync wait per instruction; peel
    extra waits onto preceding same-engine NOPs."""
    for bb in nc.main_func.blocks:
        out_list = []
        for ins in bb.instructions:
            si = ins.sync_info
            pre = []
            if si is not None and len(si.on_wait) > maxw:
                waits = list(si.on_wait)
                k = 0
                while len(waits) > maxw:
                    chunk, waits = waits[:maxw], waits[maxw:]
                    pre.append(mybir.InstNoOp(
                        name=f"{ins.name}-wsplit{k}", engine=ins.engine,
                        sync_info=mybir.SyncInfo(on_wait=chunk, on_update=[]),
                        bass_nofuse=True))
                    k += 1
                si.on_wait = waits
                ins.sync_info = si
            out_list.extend(pre)
            out_list.append(ins)
        bb.instructions = out_list


def build():
    nc = bass.Bass("TRN2", target_bir_lowering=False, debug=False,
                   num_devices=NCORES)

    def param(name, shape, dt=BF, out=False):
        return nc.declare_dram_parameter(name, shape, dt, isOutput=out)

    xT_p = param("xT", [128, DK, S])             # x[b].T (natural key order)
    xTq_p = param("xTq", [128, DK, QW])          # own 512 query rows of x[b].T
    wq_p = param("wq", [128, DK, D])             # Wq.T  [din, dout] chunked
    wk_p = param("wk", [128, DK, D])
    wv_p = param("wv", [128, DK, D])
    wp_p = param("wp", [128, DK, D])
    w1_p = param("w1", [128, DK, DFF])           # W1.T
    w2_p = param("w2", [128, DT, D])             # W2.T
    resid_p = param("resid", [128, RT, D], F32)  # x rows + bp (host-folded)
    bq_p = param("bq", [128, DK], F32)
    bk_p = param("bk", [128, DK], F32)
    bv_p = param("bv", [128, DK], F32)
    bf1_p = param("bf1", [128, DT], F32)
    bf2_p = param("bf2", [D], F32)
    g1_p = param("g1", [D], F32)
    be1_p = param("be1", [D], F32)
    g2_p = param("g2", [D], F32)
    be2_p = param("be2", [D], F32)
    maskm_p = param("maskm", [128, KC], F32)     # multiplicative mask
    out_p = param("out", [128, RT, D], F32, out=True)

    with tile.TileContext(nc) as tc:
        with tc.tile_pool(name="const", bufs=1) as const, \
             tc.tile_pool(name="persist", bufs=1) as persist:

            # ---- constants / broadcasts ----
            g1b = const.tile([128, D], F32)
            be1b = const.tile([128, D], F32)
            g2b = const.tile([128, D], F32)
            be2b = const.tile([128, D], F32)
            bf2b = const.tile([128, D], F32)
            nc.gpsimd.dma_start(g1b[:], g1_p[None, :].to_broadcast((128, D)))
            nc.gpsimd.dma_start(be1b[:], be1_p[None, :].to_broadcast((128, D)))
            nc.gpsimd.dma_start(g2b[:], g2_p[None, :].to_broadcast((128, D)))
            nc.gpsimd.dma_start(be2b[:], be2_p[None, :].to_broadcast((128, D)))
            nc.gpsimd.dma_start(bf2b[:], bf2_p[None, :].to_broadcast((128, D)))
            bq_sb = const.tile([128, DK], F32)
            bk_sb = const.tile([128, DK], F32)
            bv_sb = const.tile([128, DK], F32)
            bf1_sb = const.tile([128, DT], F32)
            maskm_sb = const.tile([128, KC], F32)
            nc.sync.dma_start(bq_sb[:], bq_p[:])
            nc.sync.dma_start(bk_sb[:], bk_p[:])
            nc.sync.dma_start(bv_sb[:], bv_p[:])
            nc.sync.dma_start(bf1_sb[:], bf1_p[:])
            nc.sync.dma_start(maskm_sb[:], maskm_p[:])
            eps_sb = const.tile([128, 1], F32)
            nc.vector.memset(eps_sb[:], EPS)
            ident = const.tile([128, 128], BF)
            make_identity(nc, ident[:])
            # preload the natural_log_exp ACT table before the first real exp
            warm_sb = const.tile([1, 1], F32)
            nc.scalar.activation(warm_sb[:], eps_sb[0:1, :], AF.Exp)
            nc.scalar.activation(warm_sb[:], eps_sb[0:1, :], AF.Ln)
            # mask broadcast along 64 free cols -> lhsT for denominator matmul
            m64_sb = const.tile([128, KC, 64], BF)
            for kc in range(KC):
                nc.vector.tensor_copy(
                    out=m64_sb[:, kc, :],
                    in_=maskm_sb[:, kc:kc + 1].to_broadcast((128, 64)))

            # ---- persistent activations (live across scope boundary) ----
            hT_sb = persist.tile([128, DK, QW], BF)    # attn out transposed
            x1res = persist.tile([128, RT, D], F32)    # LN1 out, f32 for resid
            x1T_sb = persist.tile([128, DK, QW], BF)   # LN1 out transposed
            wp_sb = persist.tile([128, DK, D], BF)     # proj weight, loaded early

            def layer_norm(xf, gamma_b, beta_b, wpool):
                """in-place LN over the free axis (768) of [128, 768] f32."""
                stats = wpool.tile([128, 3, 6], F32, tag="bnstats")
                for sg in range(3):
                    nc.vector.bn_stats(stats[:, sg, :],
                                       xf[:, sg * 256:(sg + 1) * 256])
                mv = wpool.tile([128, 2], F32, tag="bnmv")
                nc.vector.bn_aggr(mv[:], stats[:])
                lnv = wpool.tile([128, 1], F32, tag="lnv")
                nc.scalar.activation(lnv[:], mv[:, 1:2], AF.Ln, bias=eps_sb[:])
                rstd = wpool.tile([128, 1], F32, tag="rstd")
                nc.scalar.activation(rstd[:], lnv[:], AF.Exp, scale=-0.5)
                # xf = ((xf - mean) * gamma) * rstd + beta   (2 fused DVE ops)
                nc.vector.scalar_tensor_tensor(
                    out=xf, in0=xf, scalar=mv[:, 0:1], in1=gamma_b[:],
                    op0=ALU.subtract, op1=ALU.mult)
                nc.vector.scalar_tensor_tensor(
                    out=xf, in0=xf, scalar=rstd[:], in1=beta_b[:],
                    op0=ALU.mult, op1=ALU.add)

            # ============ QKV + attention (interleaved superstep) ============
            with tc.tile_pool(name="attnsc", bufs=1) as attnsc, \
                 tc.tile_pool(name="wstream", bufs=3) as wstream, \
                 tc.tile_pool(name="work", bufs=2) as work, \
                 tc.tile_pool(name="psA", bufs=2, space="PSUM") as psA, \
                 tc.tile_pool(name="psS", bufs=2, space="PSUM") as psS, \
                 tc.tile_pool(name="psPV", bufs=1, space="PSUM") as psPV:

                xTq_sb = attnsc.tile([128, DK, QW], BF)
                nc.sync.dma_start(xTq_sb[:], xTq_p[:])
                wq_sb = wstream.tile([128, DK, D], BF, tag="wproj")
                nc.sync.dma_start(wq_sb[:], wq_p[:])
                wk_sb = wstream.tile([128, DK, D], BF, tag="wproj")
                nc.sync.dma_start(wk_sb[:], wk_p[:])
                QT_sb = attnsc.tile([128, DK, QW], BF)
                KT_sb = attnsc.tile([128, DK, S], BF)
                V_sb = attnsc.tile([128, KC, D], BF)
                xT_sb = attnsc.tile([128, DK, S], BF)
                for k in range(DK):
                    nc.sync.dma_start(xT_sb[:, k, :], xT_p[:, k, :])
                wv_sb = wstream.tile([128, DK, D], BF, tag="wproj")
                nc.sync.dma_start(wv_sb[:], wv_p[:])

                # Q^T [768, 512] (own queries)
                for m in range(DK):
                    ps = psA.tile([128, QW], F32, tag="psA")
                    for k in range(DK):
                        nc.tensor.matmul(
                            ps[:], wq_sb[:, k, m * 128:(m + 1) * 128],
                            xTq_sb[:, k, :],
                            start=(k == 0), stop=(k == DK - 1))
                    nc.scalar.activation(QT_sb[:, m, :], ps[:], AF.Identity,
                                         bias=bq_sb[:, m:m + 1])

                def kt_thunks(prs):
                    """K^T m-tiles for the given head pairs, over all keys."""
                    thunks = []
                    for pr in prs:
                        for n in range(S // QW):
                            def kt_tile(pr=pr, n=n):
                                ps = psA.tile([128, QW], F32, tag="psA")
                                for k in range(DK):
                                    nc.tensor.matmul(
                                        ps[:],
                                        wk_sb[:, k, pr * 128:(pr + 1) * 128],
                                        xT_sb[:, k, n * QW:(n + 1) * QW],
                                        start=(k == 0), stop=(k == DK - 1))
                                nc.scalar.activation(
                                    KT_sb[:, pr, n * QW:(n + 1) * QW], ps[:],
                                    AF.Identity, bias=bk_sb[:, pr:pr + 1])
                            thunks.append(kt_tile)
                    return thunks

                def v_thunks(lo, hi):
                    """V columns [lo,hi) for all key rows."""
                    thunks = []
                    for rt in range(KC):
                        def v_tile(rt=rt, lo=lo, hi=hi):
                            ps = psA.tile([128, hi - lo], F32, tag="psA")
                            for k in range(DK):
                                nc.tensor.matmul(
                                    ps[:], xT_sb[:, k, rt * 128:(rt + 1) * 128],
                                    wv_sb[:, k, lo:hi],
                                    start=(k == 0), stop=(k == DK - 1))
                            nc.vector.tensor_scalar_mul(
                                out=V_sb[:, rt, lo:hi],
                                in0=ps[:], scalar1=maskm_sb[:, rt:rt + 1])
                        thunks.append(v_tile)
                    return thunks

                # minimal prolog: only K^T m-tile 0; V rows arrive just ahead
                # of each PV via double-rate filler drain during pair 0
                for t in kt_thunks((0,)):
                    t()
                filler = (v_thunks(0, 256) + kt_thunks((1, 2, 3))
                          + v_thunks(256, 512)
                          + kt_thunks((4, 5)) + v_thunks(512, 768))

                fi = 0
                for pr in range(NP):
                    if pr == 2:
                        # prefetch proj weight + residual during attention
                        nc.sync.dma_start(wp_sb[:], wp_p[:])
                        nc.sync.dma_start(x1res[:], resid_p[:])
                    # [0:512]=P@V (heads stacked 64|64), [512:1024]=denoms
                    pv = psPV.tile([128, 1024], F32, tag="pv")
                    for kc in range(KC):
                        # drain KV-production filler first: consumers below
                        # must follow their producers in program order
                        drains = 2 if pr == 0 else 1
                        for _ in range(drains):
                            if fi < len(filler):
                                filler[fi]()
                                fi += 1
                        sps = psS.tile([128, 1024], F32, tag="psS")
                        for j in range(2):
                            hp = j * 64
                            nc.tensor.matmul(
                                sps[:, j * QW:(j + 1) * QW],
                                KT_sb[hp:hp + 64, pr,
                                      kc * 128:(kc + 1) * 128],
                                QT_sb[hp:hp + 64, pr, :],
                                start=True, stop=True)
                        esb = work.tile([128, 1024], BF, tag="expS")
                        nc.scalar.activation(esb[:], sps[:], AF.Exp,
                                             scale=0.125)
                        for j in range(2):
                            h = pr * 2 + j
                            nc.tensor.matmul(
                                pv[j * 64:(j + 1) * 64, 0:QW],
                                V_sb[:, kc, h * 64:(h + 1) * 64],
                                esb[:, j * QW:(j + 1) * QW],
                                start=(kc == 0), stop=(kc == KC - 1))
                        for j in range(2):
                            nc.tensor.matmul(
                                pv[j * 64:(j + 1) * 64, QW:2 * QW],
                                m64_sb[:, kc, :],
                                esb[:, j * QW:(j + 1) * QW],
                                start=(kc == 0), stop=(kc == KC - 1))
                    # free PSUM fast: copy out accumulators, then divide
                    pvs = work.tile([128, 2, QW], F32, tag="pvs")
                    nc.vector.tensor_copy(out=pvs[:], in_=pv[:])
                    denr = work.tile([128, QW], F32, tag="denr")
                    nc.vector.reciprocal(denr[:], pvs[:, 1, :])
                    nc.vector.tensor_mul(out=hT_sb[:, pr, :],
                                         in0=pvs[:, 0, :], in1=denr[:])
                    nc.vector.tensor_scalar_add(
                        out=hT_sb[:, pr, :], in0=hT_sb[:, pr, :],
                        scalar1=bv_sb[:, pr:pr + 1])
                while fi < len(filler):
                    filler[fi]()
                    fi += 1

            # ============ out-proj + LN1 + transpose + FFN ============
            with tc.tile_pool(name="tailsc", bufs=1) as tailsc, \
                 tc.tile_pool(name="fwork", bufs=2) as fwork, \
                 tc.tile_pool(name="psM", bufs=4, space="PSUM") as psM, \
                 tc.tile_pool(name="psT", bufs=2, space="PSUM") as psT:
                w1_sb = tailsc.tile([128, DK, DFF], BF)
                nc.sync.dma_start(w1_sb[:], w1_p[:])
                w2_sb = tailsc.tile([128, DT, D], BF)
                nc.sync.dma_start(w2_sb[:], w2_p[:])
                midg = tailsc.tile([128, DT, QW], BF)

                for r in range(RT):
                    xf = x1res[:, r, :]
                    for nh in range(2):
                        ps = psM.tile([128, 384], F32, tag="psM")
                        for k in range(DK):
                            nc.tensor.matmul(
                                ps[:], hT_sb[:, k, r * 128:(r + 1) * 128],
                                wp_sb[:, k, nh * 384:(nh + 1) * 384],
                                start=(k == 0), stop=(k == DK - 1))
                        nc.vector.tensor_add(
                            out=xf[:, nh * 384:(nh + 1) * 384], in0=ps[:],
                            in1=xf[:, nh * 384:(nh + 1) * 384])
                    layer_norm(xf, g1b, be1b, fwork)
                    x1bf = fwork.tile([128, D], BF, tag="x1bf")
                    nc.vector.tensor_copy(out=x1bf[:], in_=xf)
                    for k in range(DK):
                        pt = psT.tile([128, 128], BF, tag="psT")
                        nc.tensor.transpose(
                            pt[:], x1bf[:, k * 128:(k + 1) * 128],
                            ident[:])
                        nc.vector.tensor_copy(
                            out=x1T_sb[:, k, r * 128:(r + 1) * 128],
                            in_=pt[:])

                for t in range(DT):
                    ps = psM.tile([128, QW], F32, tag="psM")
                    for k in range(DK):
                        nc.tensor.matmul(
                            ps[:], w1_sb[:, k, t * 128:(t + 1) * 128],
                            x1T_sb[:, k, :],
                            start=(k == 0), stop=(k == DK - 1))
                    nc.scalar.activation(midg[:, t, :], ps[:], AF.Gelu,
                                         bias=bf1_sb[:, t:t + 1])

                # FFN2 row-outer (w2 resident): epilogue overlaps next row
                for r in range(RT):
                    yf = fwork.tile([128, D], F32, tag="yf")
                    for nh in range(2):
                        ps = psM.tile([128, 384], F32, tag="psM")
                        for t in range(DT):
                            nc.tensor.matmul(
                                ps[:], midg[:, t, r * 128:(r + 1) * 128],
                                w2_sb[:, t, nh * 384:(nh + 1) * 384],
                                start=(t == 0), stop=(t == DT - 1))
                        nc.vector.tensor_add(
                            out=yf[:, nh * 384:(nh + 1) * 384], in0=ps[:],
                            in1=x1res[:, r, nh * 384:(nh + 1) * 384])
                    nc.vector.tensor_add(out=yf[:], in0=yf[:], in1=bf2b[:])
                    layer_norm(yf[:], g2b, be2b, fwork)
                    nc.sync.dma_start(out_p[:, r, :], yf[:])

    _split_sync_waits(nc)
    return nc


def _stage(x, mask, Wq, bq, Wk, bk, Wv, bv, Wp, bp, g1, be1, W1, bf1, W2, bf2,
           g2, be2):
    """Build per-core input maps (host-side sharding + layout)."""
    bf16 = ml_dtypes.bfloat16

    def chunkP(a):
        # [n*128, m] -> [128, n, m]
        n = a.shape[0] // 128
        return np.ascontiguousarray(
            a.reshape(n, 128, *a.shape[1:]).transpose(1, 0, 2))

    def colP(v):
        # [n*128] -> [128, n]
        return np.ascontiguousarray(v.reshape(-1, 128).T)

    wq_s = chunkP(np.ascontiguousarray(Wq.T)).astype(bf16)
    wk_s = chunkP(np.ascontiguousarray(Wk.T)).astype(bf16)
    wv_s = chunkP(np.ascontiguousarray(Wv.T)).astype(bf16)
    wp_s = chunkP(np.ascontiguousarray(Wp.T)).astype(bf16)
    w1_s = chunkP(np.ascontiguousarray(W1.T)).astype(bf16)
    w2_s = chunkP(np.ascontiguousarray(W2.T)).astype(bf16)
    bq_s, bk_s, bv_s = (colP(bq).astype(np.float32),
                        colP(bk).astype(np.float32),
                        colP(bv).astype(np.float32))
    bf1_s = colP(bf1).astype(np.float32)
    shared = dict(wq=wq_s, wk=wk_s, wv=wv_s, wp=wp_s, w1=w1_s, w2=w2_s,
                  bq=bq_s, bk=bk_s, bv=bv_s, bf1=bf1_s,
                  bf2=bf2.astype(np.float32), g1=g1.astype(np.float32),
                  be1=be1.astype(np.float32), g2=g2.astype(np.float32),
                  be2=be2.astype(np.float32))

    in_maps = []
    xT_by_batch = [chunkP(np.ascontiguousarray(x[b].T)).astype(bf16)
                   for b in range(B)]
    maskm_by_batch = [colP(mask[b].astype(np.float32)) for b in range(B)]
    for c in range(NCORES):
        b, qi = c // 4, c % 4
        xb = x[b]                                     # [2048, 768]
        rows = xb[qi * QW:(qi + 1) * QW]
        xTq = chunkP(np.ascontiguousarray(rows.T)).astype(bf16)  # [128,6,512]
        resid = chunkP((rows + bp[None, :]).astype(np.float32))  # [128,4,768]
        m = dict(shared)
        m.update(xT=xT_by_batch[b], xTq=xTq, maskm=maskm_by_batch[b],
                 resid=resid)
        in_maps.append(m)
    return in_maps


def kernel(**inputs):
    from concourse.bass_utils import run_bass_kernel_spmd
    if "nc" not in _cached:
        _cached["nc"] = build()
    nc = _cached["nc"]
    inputs = {k: np.asarray(v) for k, v in inputs.items()}
    in_maps = _stage(**inputs)
    res = run_bass_kernel_spmd(nc, in_maps, core_ids=list(range(NCORES)))
    out = np.empty((B, S, D), np.float32)
    for c in range(NCORES):
        b, qi = c // 4, c % 4
        o = res.results[c]["out"]                     # [128, 4, 768]
        out[b, qi * QW:(qi + 1) * QW] = o.transpose(1, 0, 2).reshape(QW, D)
    return out



# revision 1
# speedup vs baseline: 1.1125x; 1.1125x over previous
"""Trainium2 Bass kernel for a BERT-style transformer encoder block.

Problem: x[2,2048,768] -> attention(12 heads) + FFN(3072) block, f32 in/out.

Sharding (8 cores): sequence-parallel. Core c handles batch b=c//4 and query
rows qi=c%4 (512 rows). Each core computes K^T/V for its WHOLE batch
(duplicated 4x within the batch group — measured cheaper than an AllGather
on this fabric), does attention for its 512 queries over all 2048 keys,
then proj+LN+FFN+LN row-parallel. No collectives.

Key layout/schedule choices (compute bf16 on TensorE, f32 accumulate):
- Q^T/K^T stored [128part=dout-chunk, 6, q/k]; per-head [64,*] slices give
  natural lhsT/rhs for S^T = K @ Q^T. Head PAIRS share a 128-partition tile,
  so the two S^T matmuls use row-groups 0/64 concurrently (tile_position).
- softmax without max-subtraction (scores are O(1)); exp on ScalarE with
  the 1/sqrt(hd) folded into the activation scale; denominators via a
  mask-broadcast lhsT matmul into a second PSUM tile (col-group packing).
- attention mask folded multiplicatively into V and the denominator lhsT.
- P@V as h^T = V^T @ P^T with natural-layout V as lhsT (no transposes).
- K^T/V production for the NEXT head-pair superstep is emitted as filler
  thunks inside the attention kc-loop: PE never idles while ScalarE exps,
  and stays HAM-warm at 2.4 GHz.
- LN via bn_stats/bn_aggr; rstd = exp(-0.5*ln(var+eps)) to stay in the
  natural_log_exp activation table set (shared with attention's exp).
"""

import numpy as np
import ml_dtypes

import concourse.bass as bass
import concourse.mybir as mybir
import concourse.tile as tile
from concourse.masks import make_identity

BF = mybir.dt.bfloat16
F32 = mybir.dt.float32
AF = mybir.ActivationFunctionType
ALU = mybir.AluOpType

B, S, D, DFF, H, HD = 2, 2048, 768, 3072, 12, 64
NCORES = 8
QW = 512            # query rows per core
DK = D // 128       # 6 chunks of the model dim
DT = DFF // 128     # 24 chunks of the ffn dim
KC = S // 128       # 16 key chunks
RT = QW // 128      # 4 row tiles per core
NP = H // 2         # 6 head pairs
EPS = 1e-12

_cached = {}


def _split_sync_waits(nc, maxw=1):
    """This walrus build supports only ONE sync wait per instruction; peel
    extra waits onto preceding same-engine NOPs."""
    for bb in nc.main_func.blocks:
        out_list = []
        for ins in bb.instructions:
            si = ins.sync_info
            pre = []
            if si is not None and len(si.on_wait) > maxw:
                waits = list(si.on_wait)
                k = 0
                while len(waits) > maxw:
                    chunk, waits = waits[:maxw], waits[maxw:]
                    pre.append(mybir.InstNoOp(
                        name=f"{ins.name}-wsplit{k}", engine=ins.engine,
                        sync_info=mybir.SyncInfo(on_wait=chunk, on_update=[]),
                        bass_nofuse=True))
                    k += 1
                si.on_wait = waits
                ins.sync_info = si
            out_list.extend(pre)
            out_list.append(ins)
        bb.instructions = out_list


def build():
    nc = bass.Bass("TRN2", target_bir_lowering=False, debug=False,
                   num_devices=NCORES)

    def param(name, shape, dt=BF, out=False):
        return nc.declare_dram_parameter(name, shape, dt, isOutput=out)

    xT_p = param("xT", [128, DK, S])             # x[b].T (natural key order)
    xTq_p = param("xTq", [128, DK, QW])          # own 512 query rows of x[b].T
    wq_p = param("wq", [128, DK, D])             # Wq.T  [din, dout] chunked
    wk_p = param("wk", [128, DK, D])
    wv_p = param("wv", [128, DK, D])
    wp_p = param("wp", [128, DK, D])
    w1_p = param("w1", [128, DK, DFF])           # W1.T
    w2_p = param("w2", [128, DT, D])             # W2.T
    resid_p = param("resid", [128, RT, D], F32)  # x rows + bp (host-folded)
    bq_p = param("bq", [128, DK], F32)
    bk_p = param("bk", [128, DK], F32)
    bv_p = param("bv", [128, DK], F32)
    bf1_p = param("bf1", [128, DT], F32)
    bf2_p = param("bf2", [D], F32)
    g1_p = param("g1", [D], F32)
    be1_p = param("be1", [D], F32)
    g2_p = param("g2", [D], F32)
    be2_p = param("be2", [D], F32)
    maskm_p = param("maskm", [128, KC], F32)     # multiplicative mask
    out_p = param("out", [128, RT, D], F32, out=True)

    with tile.TileContext(nc) as tc:
        with tc.tile_pool(name="const", bufs=1) as const, \
             tc.tile_pool(name="persist", bufs=1) as persist:

            # ---- constants / broadcasts ----
            g1b = const.tile([128, D], F32)
            be1b = const.tile([128, D], F32)
            g2b = const.tile([128, D], F32)
            be2b = const.tile([128, D], F32)
            bf2b = const.tile([128, D], F32)
            nc.gpsimd.dma_start(g1b[:], g1_p[None, :].to_broadcast((128, D)))
            nc.gpsimd.dma_start(be1b[:], be1_p[None, :].to_broadcast((128, D)))
            nc.gpsimd.dma_start(g2b[:], g2_p[None, :].to_broadcast((128, D)))
            nc.gpsimd.dma_start(be2b[:], be2_p[None, :].to_broadcast((128, D)))
            nc.gpsimd.dma_start(bf2b[:], bf2_p[None, :].to_broadcast((128, D)))
            bq_sb = const.tile([128, DK], F32)
            bk_sb = const.tile([128, DK], F32)
            bv_sb = const.tile([128, DK], F32)
            bf1_sb = const.tile([128, DT], F32)
            maskm_sb = const.tile([128, KC], F32)
            nc.sync.dma_start(bq_sb[:], bq_p[:])
            nc.sync.dma_start(bk_sb[:], bk_p[:])
            nc.sync.dma_start(bv_sb[:], bv_p[:])
            nc.sync.dma_start(bf1_sb[:], bf1_p[:])
            nc.sync.dma_start(maskm_sb[:], maskm_p[:])
            eps_sb = const.tile([128, 1], F32)
            nc.vector.memset(eps_sb[:], EPS)
            ident = const.tile([128, 128], BF)
            make_identity(nc, ident[:])
            # preload the natural_log_exp ACT table before the first real exp
            warm_sb = const.tile([1, 1], F32)
            nc.scalar.activation(warm_sb[:], eps_sb[0:1, :], AF.Exp)
            nc.scalar.activation(warm_sb[:], eps_sb[0:1, :], AF.Ln)
            # mask broadcast along 64 free cols -> lhsT for denominator matmul
            m64_sb = const.tile([128, KC, 64], BF)
            for kc in range(KC):
                nc.vector.tensor_copy(
                    out=m64_sb[:, kc, :],
                    in_=maskm_sb[:, kc:kc + 1].to_broadcast((128, 64)))

            # ---- persistent activations (live across scope boundary) ----
            hT_sb = persist.tile([128, DK, QW], BF)    # attn out transposed
            x1res = persist.tile([128, RT, D], F32)    # LN1 out, f32 for resid
            x1T_sb = persist.tile([128, DK, QW], BF)   # LN1 out transposed
            wp_sb = persist.tile([128, DK, D], BF)     # proj weight, loaded early

            def layer_norm(xf, gamma_b, beta_b, wpool):
                """in-place LN over the free axis (768) of [128, 768] f32."""
                stats = wpool.tile([128, 3, 6], F32, tag="bnstats")
                for sg in range(3):
                    nc.vector.bn_stats(stats[:, sg, :],
                                       xf[:, sg * 256:(sg + 1) * 256])
                mv = wpool.tile([128, 2], F32, tag="bnmv")
                nc.vector.bn_aggr(mv[:], stats[:])
                lnv = wpool.tile([128, 1], F32, tag="lnv")
                nc.scalar.activation(lnv[:], mv[:, 1:2], AF.Ln, bias=eps_sb[:])
                rstd = wpool.tile([128, 1], F32, tag="rstd")
                nc.scalar.activation(rstd[:], lnv[:], AF.Exp, scale=-0.5)
                # xf = ((xf - mean) * gamma) * rstd + beta   (2 fused DVE ops)
                nc.vector.scalar_tensor_tensor(
                    out=xf, in0=xf, scalar=mv[:, 0:1], in1=gamma_b[:],
                    op0=ALU.subtract, op1=ALU.mult)
                nc.vector.scalar_tensor_tensor(
                    out=xf, in0=xf, scalar=rstd[:], in1=beta_b[:],
                    op0=ALU.mult, op1=ALU.add)

            # ============ QKV + attention (interleaved superstep) ============
            with tc.tile_pool(name="attnsc", bufs=1) as attnsc, \
                 tc.tile_pool(name="wstream", bufs=3) as wstream, \
                 tc.tile_pool(name="work", bufs=2) as work, \
                 tc.tile_pool(name="psA", bufs=2, space="PSUM") as psA, \
                 tc.tile_pool(name="psS", bufs=2, space="PSUM") as psS, \
                 tc.tile_pool(name="psPV", bufs=1, space="PSUM") as psPV:

                xTq_sb = attnsc.tile([128, DK, QW], BF)
                nc.sync.dma_start(xTq_sb[:], xTq_p[:])
                wq_sb = wstream.tile([128, DK, D], BF, tag="wproj")
                nc.sync.dma_start(wq_sb[:], wq_p[:])
                wk_sb = wstream.tile([128, DK, D], BF, tag="wproj")
                nc.sync.dma_start(wk_sb[:], wk_p[:])
                QT_sb = attnsc.tile([128, DK, QW], BF)
                KT_sb = attnsc.tile([128, DK, S], BF)
                V_sb = attnsc.tile([128, KC, D], BF)
                xT_sb = attnsc.tile([128, DK, S], BF)
                for k in range(DK):
                    nc.sync.dma_start(xT_sb[:, k, :], xT_p[:, k, :])
                wv_sb = wstream.tile([128, DK, D], BF, tag="wproj")
                nc.sync.dma_start(wv_sb[:], wv_p[:])

                # Q^T [768, 512] (own queries)
                for m in range(DK):
                    ps = psA.tile([128, QW], F32, tag="psA")
                    for k in range(DK):
                        nc.tensor.matmul(
                            ps[:], wq_sb[:, k, m * 128:(m + 1) * 128],
                            xTq_sb[:, k, :],
                            start=(k == 0), stop=(k == DK - 1))
                    nc.scalar.activation(QT_sb[:, m, :], ps[:], AF.Identity,
                                         bias=bq_sb[:, m:m + 1])

                def kt_thunks(prs):
                    """K^T m-tiles for the given head pairs, over all keys."""
                    thunks = []
                    for pr in prs:
                        for n in range(S // QW):
                            def kt_tile(pr=pr, n=n):
                                ps = psA.tile([128, QW], F32, tag="psA")
                                for k in range(DK):
                                    nc.tensor.matmul(
                                        ps[:],
                                        wk_sb[:, k, pr * 128:(pr + 1) * 128],
                                        xT_sb[:, k, n * QW:(n + 1) * QW],
                                        start=(k == 0), stop=(k == DK - 1))
                                nc.scalar.activation(
                                    KT_sb[:, pr, n * QW:(n + 1) * QW], ps[:],
                                    AF.Identity, bias=bk_sb[:, pr:pr + 1])
                            thunks.append(kt_tile)
                    return thunks

                def v_thunks(lo, hi):
                    """V columns [lo,hi) for all key rows."""
                    thunks = []
                    for rt in range(KC):
                        def v_tile(rt=rt, lo=lo, hi=hi):
                            ps = psA.tile([128, hi - lo], F32, tag="psA")
                            for k in range(DK):
                                nc.tensor.matmul(
                                    ps[:], xT_sb[:, k, rt * 128:(rt + 1) * 128],
                                    wv_sb[:, k, lo:hi],
                                    start=(k == 0), stop=(k == DK - 1))
                            nc.vector.tensor_scalar_mul(
                                out=V_sb[:, rt, lo:hi],
                                in0=ps[:], scalar1=maskm_sb[:, rt:rt + 1])
                        thunks.append(v_tile)
                    return thunks

                # minimal prolog: only K^T m-tile 0; V rows arrive just ahead
                # of each PV via double-rate filler drain during pair 0
                for t in kt_thunks((0,)):
                    t()
                filler = (v_thunks(0, 256) + kt_thunks((1, 2, 3))
                          + v_thunks(256, 512)
                          + kt_thunks((4, 5)) + v_thunks(512, 768))

                fi = 0
                for pr in range(NP):
                    if pr == 2:
                        # prefetch proj weight + residual during attention
                        nc.sync.dma_start(wp_sb[:], wp_p[:])
                        nc.sync.dma_start(x1res[:], resid_p[:])
                    # [0:512]=P@V (heads stacked 64|64), [512:1024]=denoms
                    pv = psPV.tile([128, 1024], F32, tag="pv")
                    for kc in range(KC):
                        # drain KV-production filler first: consumers below
                        # must follow their producers in program order
                        drains = 2 if pr == 0 else 1
                        for _ in range(drains):
                            if fi < len(filler):
                                filler[fi]()
                                fi += 1
                        sps = psS.tile([128, 1024], F32, tag="psS")
                        for j in range(2):
                            hp = j * 64
                            nc.tensor.matmul(
                                sps[:, j * QW:(j + 1) * QW],
                                KT_sb[hp:hp + 64, pr,
                                      kc * 128:(kc + 1) * 128],
                                QT_sb[hp:hp + 64, pr, :],
                                start=True, stop=True)
                        esb = work.tile([128, 1024], BF, tag="expS")
                        nc.scalar.activation(esb[:], sps[:], AF.Exp,
                                             scale=0.125)
                        for j in range(2):
                            h = pr * 2 + j
                            nc.tensor.matmul(
                                pv[j * 64:(j + 1) * 64, 0:QW],
                                V_sb[:, kc, h * 64:(h + 1) * 64],
                                esb[:, j * QW:(j + 1) * QW],
                                start=(kc == 0), stop=(kc == KC - 1))
                        for j in range(2):
                            nc.tensor.matmul(
                                pv[j * 64:(j + 1) * 64, QW:2 * QW],
                                m64_sb[:, kc, :],
                                esb[:, j * QW:(j + 1) * QW],
                                start=(kc == 0), stop=(kc == KC - 1))
                    # free PSUM fast: copy out accumulators, then divide
                    pvs = work.tile([128, 2, QW], F32, tag="pvs")
                    nc.vector.tensor_copy(out=pvs[:], in_=pv[:])
                    denr = work.tile([128, QW], F32, tag="denr")
                    nc.vector.reciprocal(denr[:], pvs[:, 1, :])
                    nc.vector.tensor_mul(out=hT_sb[:, pr, :],
                                         in0=pvs[:, 0, :], in1=denr[:])
                    nc.vector.tensor_scalar_add(
                        out=hT_sb[:, pr, :], in0=hT_sb[:, pr, :],
                        scalar1=bv_sb[:, pr:pr + 1])
                while fi < len(filler):
                    filler[fi]()
                    fi += 1

            # ============ out-proj + LN1 + transpose + FFN ============
            with tc.tile_pool(name="tailsc", bufs=1) as tailsc, \
                 tc.tile_pool(name="fwork", bufs=2) as fwork, \
                 tc.tile_pool(name="psM", bufs=4, space="PSUM") as psM, \
                 tc.tile_pool(name="psT", bufs=2, space="PSUM") as psT:
                w1_sb = tailsc.tile([128, DK, DFF], BF)
                nc.sync.dma_start(w1_sb[:], w1_p[:])
                w2_sb = tailsc.tile([128, DT, D], BF)
                nc.sync.dma_start(w2_sb[:], w2_p[:])
                midg = tailsc.tile([128, DT, QW], BF)

                for r in range(RT):
                    xf = x1res[:, r, :]
                    for nh in range(2):
                        ps = psM.tile([128, 384], F32, tag="psM")
                        for k in range(DK):
                            nc.tensor.matmul(
                                ps[:], hT_sb[:, k, r * 128:(r + 1) * 128],
                                wp_sb[:, k, nh * 384:(nh + 1) * 384],
                                start=(k == 0), stop=(k == DK - 1))
                        nc.vector.tensor_add(
                            out=xf[:, nh * 384:(nh + 1) * 384], in0=ps[:],
                            in1=xf[:, nh * 384:(nh + 1) * 384])
                    layer_norm(xf, g1b, be1b, fwork)
                    x1bf = fwork.tile([128, D], BF, tag="x1bf")
                    nc.vector.tensor_copy(out=x1bf[:], in_=xf)
                    for k in range(DK):
                        pt = psT.tile([128, 128], BF, tag="psT")
                        nc.tensor.transpose(
                            pt[:], x1bf[:, k * 128:(k + 1) * 128],
                            ident[:])
                        nc.vector.tensor_copy(
                            out=x1T_sb[:, k, r * 128:(r + 1) * 128],
                            in_=pt[:])

                for t in range(DT):
                    ps = psM.tile([128, QW], F32, tag="psM")
                    for k in range(DK):
                        nc.tensor.matmul(
                            ps[:], w1_sb[:, k, t * 128:(t + 1) * 128],
                            x1T_sb[:, k, :],
                            start=(k == 0), stop=(k == DK - 1))
                    nc.scalar.activation(midg[:, t, :], ps[:], AF.Gelu,
                                         bias=bf1_sb[:, t:t + 1])

                # FFN2 row-outer (w2 resident): epilogue overlaps next row
                for r in range(RT):
                    yf = fwork.tile([128, D], F32, tag="yf")
                    for nh in range(2):
                        ps = psM.tile([128, 384], F32, tag="psM")
                        for t in range(DT):
                            nc.tensor.matmul(
                                ps[:], midg[:, t, r * 128:(r + 1) * 128],
                                w2_sb[:, t, nh * 384:(nh + 1) * 384],
                                start=(t == 0), stop=(t == DT - 1))
                        nc.vector.tensor_add(
                            out=yf[:, nh * 384:(nh + 1) * 384], in0=ps[:],
                            in1=x1res[:, r, nh * 384:(nh + 1) * 384])
                    nc.vector.tensor_add(out=yf[:], in0=yf[:], in1=bf2b[:])
                    layer_norm(yf[:], g2b, be2b, fwork)
                    nc.sync.dma_start(out_p[:, r, :], yf[:])

    _split_sync_waits(nc)
    return nc


def _stage(x, mask, Wq, bq, Wk, bk, Wv, bv, Wp, bp, g1, be1, W1, bf1, W2, bf2,
           g2, be2):
    """Build per-core input maps (host-side sharding + layout)."""
    bf16 = ml_dtypes.bfloat16

    def chunkP(a):
        # [n*128, m] -> [128, n, m]
        n = a.shape[0] // 128
        return np.ascontiguousarray(
            a.reshape(n, 128, *a.shape[1:]).transpose(1, 0, 2))

    def colP(v):
        # [n*128] -> [128, n]
        return np.ascontiguousarray(v.reshape(-1, 128).T)

    wq_s = chunkP(np.ascontiguousarray(Wq.T)).astype(bf16)
    wk_s = chunkP(np.ascontiguousarray(Wk.T)).astype(bf16)
    wv_s = chunkP(np.ascontiguousarray(Wv.T)).astype(bf16)
    wp_s = chunkP(np.ascontiguousarray(Wp.T)).astype(bf16)
    w1_s = chunkP(np.ascontiguousarray(W1.T)).astype(bf16)
    w2_s = chunkP(np.ascontiguousarray(W2.T)).astype(bf16)
    bq_s, bk_s, bv_s = (colP(bq).astype(np.float32),
                        colP(bk).astype(np.float32),
                        colP(bv).astype(np.float32))
    bf1_s = colP(bf1).astype(np.float32)
    shared = dict(wq=wq_s, wk=wk_s, wv=wv_s, wp=wp_s, w1=w1_s, w2=w2_s,
                  bq=bq_s, bk=bk_s, bv=bv_s, bf1=bf1_s,
                  bf2=bf2.astype(np.float32), g1=g1.astype(np.float32),
                  be1=be1.astype(np.float32), g2=g2.astype(np.float32),
                  be2=be2.astype(np.float32))

    in_maps = []
    xT_by_batch = [chunkP(np.ascontiguousarray(x[b].T)).astype(bf16)
                   for b in range(B)]
    maskm_by_batch = [colP(mask[b].astype(np.float32)) for b in range(B)]
    for c in range(NCORES):
        b, qi = c // 4, c % 4
        xb = x[b]                                     # [2048, 768]
        rows = xb[qi * QW:(qi + 1) * QW]
        xTq = chunkP(np.ascontiguousarray(rows.T)).astype(bf16)  # [128,6,512]
        resid = chunkP((rows + bp[None, :]).astype(np.float32))  # [128,4,768]
        m = dict(shared)
        m.update(xT=xT_by_batch[b], xTq=xTq, maskm=maskm_by_batch[b],
                 resid=resid)
        in_maps.append(m)
    return in_maps


def kernel(**inputs):
    from concourse.bass_utils import run_bass_kernel_spmd
    if "nc" not in _cached:
        _cached["nc"] = build()
    nc = _cached["nc"]
    inputs = {k: np.asarray(v) for k, v in inputs.items()}
    in_maps = _stage(**inputs)
    res = run_bass_kernel_spmd(nc, in_maps, core_ids=list(range(NCORES)))
    out = np.empty((B, S, D), np.float32)
    for c in range(NCORES):
        b, qi = c // 4, c % 4
        o = res.results[c]["out"]                     # [128, 4, 768]
        out[b, qi * QW:(qi + 1) * QW] = o.transpose(1, 0, 2).reshape(QW, D)
    return out



# revision 3
# speedup vs baseline: 1.1264x; 1.0126x over previous
"""Trainium2 Bass kernel for a BERT-style transformer encoder block.

Problem: x[2,2048,768] -> attention(12 heads) + FFN(3072) block, f32 in/out.

Sharding (8 cores): sequence-parallel. Core c handles batch b=c//4 and query
rows qi=c%4 (512 rows). Each core computes K^T/V for its WHOLE batch
(duplicated 4x within the batch group — measured cheaper than an AllGather
on this fabric), does attention for its 512 queries over all 2048 keys,
then proj+LN+FFN+LN row-parallel. No collectives.

Key layout/schedule choices:
- Attention-side GEMMs (Q/K/V projections, out-proj) run in fp8e4 with
  perf_mode=DoubleRow (pairs of 128-contraction chunks per pass, ~1.7x).
  Weights are host-scaled x64 to stay in fp8 normal range; Q^T/K^T hold
  64x values in bf16 (bias host-scaled too) and the combined 1/4096 rides
  the exp scale. V epilogue multiplies by mask/64; the denominator lhsT is
  mask/64 so the reciprocal directly yields 64/den and h^T lands at 64x,
  in-range for its fp8 store feeding the DoubleRow out-proj.
- FFN stays bf16: fp8 weight-quantization noise passes ~1:1 into the FFN
  output (coherent per-output-channel), measured 1.3e-2 final rel err vs
  1.2e-3 with fp8 confined to the attention side (h is tiny vs residual).
- Q^T/K^T stored [128part=dout-chunk, 6, q/k]; per-head [64,*] slices give
  natural lhsT/rhs for S^T = K @ Q^T. Head PAIRS share a 128-partition tile,
  so the two S^T matmuls use row-groups 0/64 concurrently; PV and the
  mask-broadcast denominator matmuls pack into column groups.
- softmax without max-subtraction; exp on ScalarE with all scales folded in;
  Q/K bias epilogues on DVE to keep ScalarE for exp (its real bottleneck).
- K^T/V production for later head-pairs is emitted as filler thunks inside
  the attention kc-loop so PE never idles while ScalarE exps.
- DMA order: critical-path tensors (xTq, wq, wk, xT, wv) first; const
  broadcasts after; wp/resid/w1 prefetched during attention, w2 at tail
  start. reciprocal_approx_fast for the softmax denominators.
- LN via bn_stats/bn_aggr; rstd = exp(-0.5*ln(var+eps)) to stay in the
  natural_log_exp activation table set (shared with attention's exp).
"""

import numpy as np
import ml_dtypes

import concourse.bass as bass
import concourse.mybir as mybir
import concourse.tile as tile
from concourse.masks import make_identity

BF = mybir.dt.bfloat16
F32 = mybir.dt.float32
F8 = mybir.dt.float8e4
AF = mybir.ActivationFunctionType
ALU = mybir.AluOpType
DR = mybir.MatmulPerfMode.DoubleRow

B, S, D, DFF, H, HD = 2, 2048, 768, 3072, 12, 64
NCORES = 8
QW = 512            # query rows per core
DK = D // 128       # 6 chunks of the model dim
DT = DFF // 128     # 24 chunks of the ffn dim
KC = S // 128       # 16 key chunks
RT = QW // 128      # 4 row tiles per core
NP = H // 2         # 6 head pairs
EPS = 1e-12
WS = 64.0           # host-side weight scale for fp8
ES = 0.125 / (WS * WS)   # exp scale: 1/sqrt(hd) and the two 64x Q/K scales

_cached = {}


def _split_sync_waits(nc, maxw=1):
    """This walrus build supports only ONE sync wait per instruction; peel
    extra waits onto preceding same-engine NOPs."""
    for bb in nc.main_func.blocks:
        out_list = []
        for ins in bb.instructions:
            si = ins.sync_info
            pre = []
            if si is not None and len(si.on_wait) > maxw:
                waits = list(si.on_wait)
                k = 0
                while len(waits) > maxw:
                    chunk, waits = waits[:maxw], waits[maxw:]
                    pre.append(mybir.InstNoOp(
                        name=f"{ins.name}-wsplit{k}", engine=ins.engine,
                        sync_info=mybir.SyncInfo(on_wait=chunk, on_update=[]),
                        bass_nofuse=True))
                    k += 1
                si.on_wait = waits
                ins.sync_info = si
            out_list.extend(pre)
            out_list.append(ins)
        bb.instructions = out_list


def build():
    nc = bass.Bass("TRN2", target_bir_lowering=False, debug=False,
                   num_devices=NCORES)

    def param(name, shape, dt=BF, out=False):
        return nc.declare_dram_parameter(name, shape, dt, isOutput=out)

    xT_p = param("xT", [128, DK, S], F8)         # x[b].T (natural key order)
    xTq_p = param("xTq", [128, DK, QW], F8)      # own 512 query rows of x[b].T
    wq_p = param("wq", [128, DK, D], F8)         # 64*Wq.T  [din, dout] chunked
    wk_p = param("wk", [128, DK, D], F8)
    wv_p = param("wv", [128, DK, D], F8)
    wp_p = param("wp", [128, DK, D], F8)
    w1_p = param("w1", [128, DK, DFF])           # W1.T (bf16)
    w2_p = param("w2", [128, DT, D])             # W2.T (bf16)
    resid_p = param("resid", [128, RT, D], F32)  # x rows + bp (host-folded)
    bq_p = param("bq", [128, DK], F32)           # 64*bq
    bk_p = param("bk", [128, DK], F32)
    bv_p = param("bv", [128, DK], F32)
    bf1_p = param("bf1", [128, DT], F32)
    bf2_p = param("bf2", [D], F32)
    g1_p = param("g1", [D], F32)
    be1_p = param("be1", [D], F32)
    g2_p = param("g2", [D], F32)
    be2_p = param("be2", [D], F32)
    maskv_p = param("maskv", [128, KC], F32)     # mask/64 (multiplicative)
    out_p = param("out", [128, RT, D], F32, out=True)

    with tile.TileContext(nc) as tc:
        with tc.tile_pool(name="const", bufs=1) as const, \
             tc.tile_pool(name="persist", bufs=1) as persist:

            # ---- const tiles (DMAs emitted later, off the critical path) ----
            g1b = const.tile([128, D], F32)
            be1b = const.tile([128, D], F32)
            g2b = const.tile([128, D], F32)
            be2b = const.tile([128, D], F32)
            bf2b = const.tile([128, D], F32)
            bq_sb = const.tile([128, DK], F32)
            bk_sb = const.tile([128, DK], F32)
            bv_sb = const.tile([128, DK], F32)
            bf1_sb = const.tile([128, DT], F32)
            maskv_sb = const.tile([128, KC], F32)
            eps_sb = const.tile([128, 1], F32)
            ident = const.tile([128, 128], BF)
            warm_sb = const.tile([1, 1], F32)
            m64_sb = const.tile([128, KC, 64], BF)

            # ---- persistent activations (live across scope boundary) ----
            hT_sb = persist.tile([128, DK, QW], F8)    # 64*(h+bv), fp8
            x1res = persist.tile([128, RT, D], F32)    # LN1 out, f32 for resid
            x1T_sb = persist.tile([128, DK, QW], BF)   # LN1 out transposed
            wp_sb = persist.tile([128, DK, D], F8)     # proj weight
            w1_sb = persist.tile([128, DK, DFF], BF)   # ffn1 weight, prefetched

            def layer_norm(xf, gamma_b, beta_b, wpool):
                """in-place LN over the free axis (768) of [128, 768] f32."""
                stats = wpool.tile([128, 3, 6], F32, tag="bnstats")
                for sg in range(3):
                    nc.vector.bn_stats(stats[:, sg, :],
                                       xf[:, sg * 256:(sg + 1) * 256])
                mv = wpool.tile([128, 2], F32, tag="bnmv")
                nc.vector.bn_aggr(mv[:], stats[:])
                lnv = wpool.tile([128, 1], F32, tag="lnv")
                nc.scalar.activation(lnv[:], mv[:, 1:2], AF.Ln, bias=eps_sb[:])
                rstd = wpool.tile([128, 1], F32, tag="rstd")
                nc.scalar.activation(rstd[:], lnv[:], AF.Exp, scale=-0.5)
                # xf = ((xf - mean) * gamma) * rstd + beta   (2 fused DVE ops)
                nc.vector.scalar_tensor_tensor(
                    out=xf, in0=xf, scalar=mv[:, 0:1], in1=gamma_b[:],
                    op0=ALU.subtract, op1=ALU.mult)
                nc.vector.scalar_tensor_tensor(
                    out=xf, in0=xf, scalar=rstd[:], in1=beta_b[:],
                    op0=ALU.mult, op1=ALU.add)

            # ============ QKV + attention (interleaved superstep) ============
            with tc.tile_pool(name="attnsc", bufs=1) as attnsc, \
                 tc.tile_pool(name="wstream", bufs=3) as wstream, \
                 tc.tile_pool(name="work", bufs=2) as work, \
                 tc.tile_pool(name="psA", bufs=2, space="PSUM") as psA, \
                 tc.tile_pool(name="psS", bufs=2, space="PSUM") as psS, \
                 tc.tile_pool(name="psPV", bufs=1, space="PSUM") as psPV:

                # critical-path DMAs first
                xTq_sb = attnsc.tile([128, DK, QW], F8)
                nc.sync.dma_start(xTq_sb[:], xTq_p[:])
                wq_sb = wstream.tile([128, DK, D], F8, tag="wproj")
                nc.sync.dma_start(wq_sb[:], wq_p[:])
                wk_sb = wstream.tile([128, DK, D], F8, tag="wproj")
                nc.sync.dma_start(wk_sb[:], wk_p[:])
                QT_sb = attnsc.tile([128, DK, QW], BF)
                KT_sb = attnsc.tile([128, DK, S], BF)
                V_sb = attnsc.tile([128, KC, D], BF)
                xT_sb = attnsc.tile([128, DK, S], F8)
                for k in range(DK):
                    nc.sync.dma_start(xT_sb[:, k, :], xT_p[:, k, :])
                wv_sb = wstream.tile([128, DK, D], F8, tag="wproj")
                nc.sync.dma_start(wv_sb[:], wv_p[:])
                nc.sync.dma_start(bq_sb[:], bq_p[:])
                nc.sync.dma_start(bk_sb[:], bk_p[:])
                nc.sync.dma_start(bv_sb[:], bv_p[:])
                nc.sync.dma_start(maskv_sb[:], maskv_p[:])

                # non-critical consts (needed only by the tail)
                nc.gpsimd.dma_start(g1b[:], g1_p[None, :].to_broadcast((128, D)))
                nc.gpsimd.dma_start(be1b[:], be1_p[None, :].to_broadcast((128, D)))
                nc.gpsimd.dma_start(g2b[:], g2_p[None, :].to_broadcast((128, D)))
                nc.gpsimd.dma_start(be2b[:], be2_p[None, :].to_broadcast((128, D)))
                nc.gpsimd.dma_start(bf2b[:], bf2_p[None, :].to_broadcast((128, D)))
                nc.sync.dma_start(bf1_sb[:], bf1_p[:])
                nc.vector.memset(eps_sb[:], EPS)
                make_identity(nc, ident[:])
                # preload the natural_log_exp ACT table before the first exp
                nc.scalar.activation(warm_sb[:], eps_sb[0:1, :], AF.Exp)
                nc.scalar.activation(warm_sb[:], eps_sb[0:1, :], AF.Ln)
                # mask/64 broadcast along 64 cols -> lhsT for denominators
                for kc in range(KC):
                    nc.vector.tensor_copy(
                        out=m64_sb[:, kc, :],
                        in_=maskv_sb[:, kc:kc + 1].to_broadcast((128, 64)))

                # Q^T [768, 512] (own queries); fp8 DoubleRow, DVE epilogue
                for m in range(DK):
                    ps = psA.tile([128, QW], F32, tag="psA")
                    for k in range(0, DK, 2):
                        nc.tensor.matmul(
                            ps[:], wq_sb[:, k:k + 2, m * 128:(m + 1) * 128],
                            xTq_sb[:, k:k + 2, :],
                            start=(k == 0), stop=(k == DK - 2), perf_mode=DR)
                    nc.vector.tensor_scalar_add(
                        out=QT_sb[:, m, :], in0=ps[:],
                        scalar1=bq_sb[:, m:m + 1])

                def kt_thunks(prs):
                    """K^T m-tiles for the given head pairs, over all keys."""
                    thunks = []
                    for pr in prs:
                        for n in range(S // QW):
                            def kt_tile(pr=pr, n=n):
                                ps = psA.tile([128, QW], F32, tag="psA")
                                for k in range(0, DK, 2):
                                    nc.tensor.matmul(
                                        ps[:],
                                        wk_sb[:, k:k + 2,
                                              pr * 128:(pr + 1) * 128],
                                        xT_sb[:, k:k + 2,
                                              n * QW:(n + 1) * QW],
                                        start=(k == 0), stop=(k == DK - 2),
                                        perf_mode=DR)
                                nc.vector.tensor_scalar_add(
                                    out=KT_sb[:, pr, n * QW:(n + 1) * QW],
                                    in0=ps[:], scalar1=bk_sb[:, pr:pr + 1])
                            thunks.append(kt_tile)
                    return thunks

                def v_thunks(lo, hi):
                    """V columns [lo,hi) for all key rows."""
                    thunks = []
                    for rt in range(KC):
                        def v_tile(rt=rt, lo=lo, hi=hi):
                            ps = psA.tile([128, hi - lo], F32, tag="psA")
                            for k in range(0, DK, 2):
                                nc.tensor.matmul(
                                    ps[:],
                                    xT_sb[:, k:k + 2, rt * 128:(rt + 1) * 128],
                                    wv_sb[:, k:k + 2, lo:hi],
                                    start=(k == 0), stop=(k == DK - 2),
                                    perf_mode=DR)
                            nc.vector.tensor_scalar_mul(
                                out=V_sb[:, rt, lo:hi],
                                in0=ps[:], scalar1=maskv_sb[:, rt:rt + 1])
                        thunks.append(v_tile)
                    return thunks

                # minimal prolog: only K^T m-tile 0; V rows arrive just ahead
                # of each PV via double-rate filler drain during pair 0
                for t in kt_thunks((0,)):
                    t()
                filler = (v_thunks(0, 256) + kt_thunks((1, 2, 3))
                          + v_thunks(256, 512)
                          + kt_thunks((4, 5)) + v_thunks(512, 768))

                fi = 0
                for pr in range(NP):
                    if pr == 1:
                        # prefetch proj weight + residual during attention
                        nc.sync.dma_start(wp_sb[:], wp_p[:])
                        nc.sync.dma_start(x1res[:], resid_p[:])
                    if pr == 2:
                        # prefetch the (large, bf16) FFN1 weight too
                        nc.sync.dma_start(w1_sb[:], w1_p[:])
                    # [0:512]=P@V (heads stacked 64|64), [512:1024]=denoms
                    pv = psPV.tile([128, 1024], F32, tag="pv")
                    for kc in range(KC):
                        # drain KV-production filler first: consumers below
                        # must follow their producers in program order
                        drains = 2 if pr == 0 else 1
                        for _ in range(drains):
                            if fi < len(filler):
                                filler[fi]()
                                fi += 1
                        sps = psS.tile([128, 1024], F32, tag="psS")
                        for j in range(2):
                            hp = j * 64
                            nc.tensor.matmul(
                                sps[:, j * QW:(j + 1) * QW],
                                KT_sb[hp:hp + 64, pr,
                                      kc * 128:(kc + 1) * 128],
                                QT_sb[hp:hp + 64, pr, :],
                                start=True, stop=True)
                        esb = work.tile([128, 1024], BF, tag="expS")
                        nc.scalar.activation(esb[:], sps[:], AF.Exp,
                                             scale=ES)
                        for j in range(2):
                            h = pr * 2 + j
                            nc.tensor.matmul(
                                pv[j * 64:(j + 1) * 64, 0:QW],
                                V_sb[:, kc, h * 64:(h + 1) * 64],
                                esb[:, j * QW:(j + 1) * QW],
                                start=(kc == 0), stop=(kc == KC - 1))
                        for j in range(2):
                            nc.tensor.matmul(
                                pv[j * 64:(j + 1) * 64, QW:2 * QW],
                                m64_sb[:, kc, :],
                                esb[:, j * QW:(j + 1) * QW],
                                start=(kc == 0), stop=(kc == KC - 1))
                    # free PSUM fast: copy out accumulators, then divide.
                    # den lhsT was mask/64, so recip is 64/den and hT = 64*h.
                    pvs = work.tile([128, 2, QW], F32, tag="pvs")
                    nc.vector.tensor_copy(out=pvs[:], in_=pv[:])
                    # 1/den = exp(-ln(den)) on ScalarE: same act table set as
                    # the attention exp, and DVE's iterative divide is ~2.4x
                    # slower than these two table ops.
                    lnd = work.tile([128, QW], F32, tag="lnd")
                    nc.scalar.activation(lnd[:], pvs[:, 1, :], AF.Ln)
                    denr = work.tile([128, QW], F32, tag="denr")
                    nc.scalar.activation(denr[:], lnd[:], AF.Exp, scale=-1.0)
                    nc.vector.tensor_mul(out=hT_sb[:, pr, :],
                                         in0=pvs[:, 0, :], in1=denr[:])
                    nc.vector.tensor_scalar_add(
                        out=hT_sb[:, pr, :], in0=hT_sb[:, pr, :],
                        scalar1=bv_sb[:, pr:pr + 1])
                while fi < len(filler):
                    filler[fi]()
                    fi += 1

            # ============ out-proj + LN1 + transpose + FFN ============
            with tc.tile_pool(name="tailsc", bufs=1) as tailsc, \
                 tc.tile_pool(name="fwork", bufs=2) as fwork, \
                 tc.tile_pool(name="psM", bufs=4, space="PSUM") as psM, \
                 tc.tile_pool(name="psT", bufs=2, space="PSUM") as psT:
                w2_sb = tailsc.tile([128, DT, D], BF)
                nc.sync.dma_start(w2_sb[:], w2_p[:])
                midg = tailsc.tile([128, DT, QW], BF)

                for r in range(RT):
                    xf = x1res[:, r, :]
                    for nh in range(2):
                        ps = psM.tile([128, 384], F32, tag="psM")
                        for k in range(0, DK, 2):
                            nc.tensor.matmul(
                                ps[:],
                                hT_sb[:, k:k + 2, r * 128:(r + 1) * 128],
                                wp_sb[:, k:k + 2, nh * 384:(nh + 1) * 384],
                                start=(k == 0), stop=(k == DK - 2),
                                perf_mode=DR)
                        # psum holds 4096*(h+bv)@Wp; rescale + residual add
                        nc.vector.scalar_tensor_tensor(
                            out=xf[:, nh * 384:(nh + 1) * 384], in0=ps[:],
                            scalar=1.0 / (WS * WS),
                            in1=xf[:, nh * 384:(nh + 1) * 384],
                            op0=ALU.mult, op1=ALU.add)
                    layer_norm(xf, g1b, be1b, fwork)
                    x1bf = fwork.tile([128, D], BF, tag="x1bf")
                    nc.vector.tensor_copy(out=x1bf[:], in_=xf)
                    for k in range(DK):
                        pt = psT.tile([128, 128], BF, tag="psT")
                        nc.tensor.transpose(
                            pt[:], x1bf[:, k * 128:(k + 1) * 128],
                            ident[:])
                        nc.vector.tensor_copy(
                            out=x1T_sb[:, k, r * 128:(r + 1) * 128],
                            in_=pt[:])

                for t in range(DT):
                    ps = psM.tile([128, QW], F32, tag="psM")
                    for k in range(DK):
                        nc.tensor.matmul(
                            ps[:], w1_sb[:, k, t * 128:(t + 1) * 128],
                            x1T_sb[:, k, :],
                            start=(k == 0), stop=(k == DK - 1))
                    nc.scalar.activation(midg[:, t, :], ps[:], AF.Gelu,
                                         bias=bf1_sb[:, t:t + 1])

                # FFN2 row-outer (w2 resident): epilogue overlaps next row
                for r in range(RT):
                    yf = fwork.tile([128, D], F32, tag="yf")
                    for nh in range(2):
                        ps = psM.tile([128, 384], F32, tag="psM")
                        for t in range(DT):
                            nc.tensor.matmul(
                                ps[:], midg[:, t, r * 128:(r + 1) * 128],
                                w2_sb[:, t, nh * 384:(nh + 1) * 384],
                                start=(t == 0), stop=(t == DT - 1))
                        nc.vector.tensor_add(
                            out=yf[:, nh * 384:(nh + 1) * 384], in0=ps[:],
                            in1=x1res[:, r, nh * 384:(nh + 1) * 384])
                    nc.vector.tensor_add(out=yf[:], in0=yf[:], in1=bf2b[:])
                    layer_norm(yf[:], g2b, be2b, fwork)
                    nc.sync.dma_start(out_p[:, r, :], yf[:])

    _split_sync_waits(nc)
    return nc


def _stage(x, mask, Wq, bq, Wk, bk, Wv, bv, Wp, bp, g1, be1, W1, bf1, W2, bf2,
           g2, be2):
    """Build per-core input maps (host-side sharding + layout)."""
    bf16 = ml_dtypes.bfloat16
    f8 = ml_dtypes.float8_e4m3

    def chunkP(a):
        # [n*128, m] -> [128, n, m]
        n = a.shape[0] // 128
        return np.ascontiguousarray(
            a.reshape(n, 128, *a.shape[1:]).transpose(1, 0, 2))

    def colP(v):
        # [n*128] -> [128, n]
        return np.ascontiguousarray(v.reshape(-1, 128).T)

    wq_s = chunkP(np.ascontiguousarray(WS * Wq.T)).astype(f8)
    wk_s = chunkP(np.ascontiguousarray(WS * Wk.T)).astype(f8)
    wv_s = chunkP(np.ascontiguousarray(WS * Wv.T)).astype(f8)
    wp_s = chunkP(np.ascontiguousarray(WS * Wp.T)).astype(f8)
    w1_s = chunkP(np.ascontiguousarray(W1.T)).astype(bf16)
    w2_s = chunkP(np.ascontiguousarray(W2.T)).astype(bf16)
    bq_s, bk_s, bv_s = (colP(WS * bq).astype(np.float32),
                        colP(WS * bk).astype(np.float32),
                        colP(WS * bv).astype(np.float32))
    bf1_s = colP(bf1).astype(np.float32)
    shared = dict(wq=wq_s, wk=wk_s, wv=wv_s, wp=wp_s, w1=w1_s, w2=w2_s,
                  bq=bq_s, bk=bk_s, bv=bv_s, bf1=bf1_s,
                  bf2=bf2.astype(np.float32), g1=g1.astype(np.float32),
                  be1=be1.astype(np.float32), g2=g2.astype(np.float32),
                  be2=be2.astype(np.float32))

    in_maps = []
    xT_by_batch = [chunkP(np.ascontiguousarray(x[b].T)).astype(f8)
                   for b in range(B)]
    maskv_by_batch = [colP((mask[b] / WS).astype(np.float32))
                      for b in range(B)]
    for c in range(NCORES):
        b, qi = c // 4, c % 4
        xb = x[b]                                     # [2048, 768]
        rows = xb[qi * QW:(qi + 1) * QW]
        xTq = chunkP(np.ascontiguousarray(rows.T)).astype(f8)    # [128,6,512]
        resid = chunkP((rows + bp[None, :]).astype(np.float32))  # [128,4,768]
        m = dict(shared)
        m.update(xT=xT_by_batch[b], xTq=xTq, maskv=maskv_by_batch[b],
                 resid=resid)
        in_maps.append(m)
    return in_maps


def kernel(**inputs):
    from concourse.bass_utils import run_bass_kernel_spmd
    if "nc" not in _cached:
        _cached["nc"] = build()
    nc = _cached["nc"]
    inputs = {k: np.asarray(v) for k, v in inputs.items()}
    in_maps = _stage(**inputs)
    res = run_bass_kernel_spmd(nc, in_maps, core_ids=list(range(NCORES)))
    out = np.empty((B, S, D), np.float32)
    for c in range(NCORES):
        b, qi = c // 4, c % 4
        o = res.results[c]["out"]                     # [128, 4, 768]
        out[b, qi * QW:(qi + 1) * QW] = o.transpose(1, 0, 2).reshape(QW, D)
    return out
